# revision 4
# baseline (speedup 1.0000x reference)
"""Bidirectional RoPE self-attention (Q is both query and key) on 8 trn2 cores.

Math (per (b,h) pair, T=1024, N=256):
    QR = rope(Q); S = QR @ QR.T / 16; out = softmax(S) @ V

Device strategy:
  - 96 (b,h) pairs sharded 12-per-core (batch/head parallel, no comm).
  - Host pre-transposes Q to [N, T] layout with even/odd channel
    deinterleave so RoPE needs no partition shuffles: channels [0::2] in
    tile0, [1::2] in tile1; rope is 6 aligned elementwise DVE ops using
    host-precomputed cos/sin tables (scaled by 1/4 so scores come out
    pre-divided by 16).
  - scores: fp32r matmuls (full PE rate at moving dim >= 256), contraction
    over the 2 channel tiles, PSUM tiles [128, 1024] (2 banks).
  - exp: one ScalarE activation per t-tile, PSUM -> SBUF, with accum_out
    producing the softmax row-sum Z for free (no max-subtraction needed:
    scores/16 <= ~22 so exp fits fp32 comfortably).
  - attn @ V: scores are symmetric, so the stored E tiles [t, s] serve as
    their own transpose: lhsT = E_tile_c[:, t-slice]. fp32r again.
  - out = psum * (1/Z) per-partition scalar on DVE, then DMA out.
"""

import os
import sys
from contextlib import ExitStack

import numpy as np

import concourse.bacc as bacc
import concourse.bass as bass
import concourse.tile as tile
from concourse import mybir

B, NH, T, N = 8, 12, 1024, 256
NCORES = 8
PAIRS = B * NH // NCORES  # 12 (b,h) pairs per core
F32 = mybir.dt.float32
F32R = mybir.dt.float32r
EXP = mybir.ActivationFunctionType.Exp

NTT = T // 128  # 8 t-tiles per pair


def build_nc(pairs=PAIRS):
    nc = bacc.Bacc("TRN2", target_bir_lowering=False, debug=False,
                   enable_asserts=False)

    qt = nc.dram_tensor("qt", [pairs, 2, 128, T], F32, kind="ExternalInput")
    v = nc.dram_tensor("v", [pairs, NTT, 128, N], F32R, kind="ExternalInput")
    cs = nc.dram_tensor("cs", [2, 128, T], F32, kind="ExternalInput")
    out = nc.dram_tensor("out", [pairs, NTT, 128, N], F32, kind="ExternalOutput")

    with tile.TileContext(nc) as tc, ExitStack() as ctx:
        cpool = ctx.enter_context(tc.tile_pool(name="cs", bufs=1))
        qpool = ctx.enter_context(tc.tile_pool(name="q", bufs=4))
        tpool = ctx.enter_context(tc.tile_pool(name="tmp", bufs=4))
        qrpool = ctx.enter_context(tc.tile_pool(name="qr", bufs=4))
        epool = ctx.enter_context(tc.tile_pool(name="e", bufs=16))
        vpool = ctx.enter_context(tc.tile_pool(name="v", bufs=16))
        opool = ctx.enter_context(tc.tile_pool(name="o", bufs=8))
        zpool = ctx.enter_context(tc.tile_pool(name="z", bufs=4))
        ps_s = ctx.enter_context(tc.tile_pool(name="ps_s", bufs=2, space="PSUM"))
        ps_o = ctx.enter_context(tc.tile_pool(name="ps_o", bufs=2, space="PSUM"))

        ctile = cpool.tile([128, T], F32, tag="c")
        stile = cpool.tile([128, T], F32, tag="s")
        nc.sync.dma_start(ctile[:], cs[0])
        nc.sync.dma_start(stile[:], cs[1])

        for p in range(pairs):
            q0 = qpool.tile([128, T], F32)
            nc.sync.dma_start(q0[:], qt[p, 0])
            q1 = qpool.tile([128, T], F32)
            nc.sync.dma_start(q1[:], qt[p, 1])

            # rope: qr0 = q0*C - q1*S ; qr1 = q1*C + q0*S   (C,S carry 1/4)
            ta = tpool.tile([128, T], F32, tag="ta")
            tb = tpool.tile([128, T], F32, tag="tb")
            nc.vector.tensor_mul(ta[:], q0[:], ctile[:])
            nc.vector.tensor_mul(tb[:], q1[:], stile[:])
            qr0 = qrpool.tile([128, T], F32R)
            nc.vector.tensor_sub(qr0[:], ta[:], tb[:])
            tc2 = tpool.tile([128, T], F32, tag="ta")
            td = tpool.tile([128, T], F32, tag="tb")
            nc.vector.tensor_mul(tc2[:], q1[:], ctile[:])
            nc.vector.tensor_mul(td[:], q0[:], stile[:])
            qr1 = qrpool.tile([128, T], F32R)
            nc.vector.tensor_add(qr1[:], tc2[:], td[:])
            qrs = (qr0, qr1)

            # V tiles for the second matmul
            vt = []
            for c in range(NTT):
                vtile = vpool.tile([128, N], F32R)
                nc.sync.dma_start(vtile[:], v[p, c])
                vt.append(vtile)

            # scores + exp (+row-sum Z) per t-tile
            zacc = zpool.tile([128, NTT], F32, tag="zacc")
            et = []
            for tt in range(NTT):
                ps = ps_s.tile([128, T], F32)
                for sc in range(T // 512):
                    for k in range(2):
                        nc.tensor.matmul(
                            ps[:, sc * 512:(sc + 1) * 512],
                            qrs[k][:, tt * 128:(tt + 1) * 128],
                            qrs[k][:, sc * 512:(sc + 1) * 512],
                            start=(k == 0), stop=(k == 1),
                        )
                e = epool.tile([128, T], F32R)
                nc.scalar.activation(e[:], ps[:], EXP,
                                     accum_out=zacc[:, tt:tt + 1])
                et.append(e)

            zrec = zpool.tile([128, NTT], F32, tag="zrec")
            nc.vector.reciprocal(zrec[:], zacc[:])

            # out[t-tile] = (sum_c E[c-tile][:, t-slice].T @ V[c-tile]) / Z
            for tt in range(NTT):
                po = ps_o.tile([128, N], F32)
                for c in range(NTT):
                    nc.tensor.matmul(
                        po[:],
                        et[c][:, tt * 128:(tt + 1) * 128],
                        vt[c][:],
                        start=(c == 0), stop=(c == NTT - 1),
                    )
                o = opool.tile([128, N], F32)
                nc.vector.tensor_scalar_mul(o[:], po[:], zrec[:, tt:tt + 1])
                nc.sync.dma_start(out[p, tt], o[:])

    nc.compile()
    return nc


def host_prep(Q, V, freqs):
    """Returns (in_maps, meta) for the 8 cores."""
    Q = np.ascontiguousarray(np.asarray(Q), dtype=np.float32)
    V = np.ascontiguousarray(np.asarray(V), dtype=np.float32)
    freqs = np.asarray(freqs, dtype=np.float32)

    # cos/sin tables in [channel-pair, t] layout, scaled by 1/4.
    half = freqs.reshape(-1)[0::2]  # [128] cycles-per-step
    t_col = np.arange(T, dtype=np.float32).reshape(T, 1)
    phases = t_col * half.reshape(1, 128)  # [T, 128] fp32
    ang = np.mod(phases, np.float32(1.0)) * np.float32(2.0 * np.pi)
    C = (np.cos(ang).astype(np.float32) * np.float32(0.25)).T  # [128, T]
    S = (np.sin(ang).astype(np.float32) * np.float32(0.25)).T
    cs_np = np.ascontiguousarray(np.stack([C, S]))  # [2, 128, T]

    G = B * NH
    Qg = Q.reshape(G, T, N)
    QT = np.empty((G, 2, 128, T), np.float32)
    QT[:, 0] = Qg[:, :, 0::2].transpose(0, 2, 1)  # even channels
    QT[:, 1] = Qg[:, :, 1::2].transpose(0, 2, 1)  # odd channels
    Vg = V.reshape(G, NTT, 128, N)

    in_maps = []
    for c in range(NCORES):
        sl = slice(c * PAIRS, (c + 1) * PAIRS)
        in_maps.append({"qt": QT[sl], "v": Vg[sl], "cs": cs_np})
    return in_maps


_CACHED_NC = None


def kernel(Q, V, freqs):
    global _CACHED_NC
    from concourse.bass_utils import run_bass_kernel_spmd

    in_maps = host_prep(Q, V, freqs)
    if _CACHED_NC is None:
        _CACHED_NC = build_nc()
    res = run_bass_kernel_spmd(_CACHED_NC, in_maps, list(range(NCORES)))
    outs = [res.results[c]["out"].reshape(PAIRS, T, N) for c in range(NCORES)]
    return np.concatenate(outs).reshape(B, NH, T, N).astype(np.float32, copy=False)
